# revision 9
# baseline (speedup 1.0000x reference)
"""CP-LSTM Trainium2 kernel (8 NeuronCores, SPMD).

Strategy:
  - The recurrent scan is replicated on every core with the full batch
    (B=32): per-step matmul cost on the PE is dominated by streaming the
    big static operands (a, ct), which is batch-size independent, so
    data-parallel sharding of the scan buys nothing -- replication
    removes all cross-core communication instead.
  - The embedding gather, x@b precompute and the scan produce hidden
    states transposed in SBUF; the decoder GEMM is tensor-parallel over
    the vocab (each core owns a 4000-row slice of dec_w / dec_b) and its
    matmuls are interleaved with the scan so the TensorEngine stays
    saturated.
  - Everything on-device runs in bf16 (fp32 PSUM accumulation, fp32
    cell state); layouts use a "folded" (128, 256) form of (B=32, 1024)
    tensors [partition 32*j+b <-> column quarter j] so elementwise work
    uses all 128 lanes, enabled by col-group-tiled matmuls
    (tile_position) that write 32-partition PSUM slices.

Layouts (device):
  B-fold  (128,256): [32j+b, q] = X[b, 256j+q]        (over 4R or H)
  hT E/O  (128,128): E[p, 32j+b] = h[b, 256j+p], O: +128 on H
  token order: token = 32*t + b; token tile k = steps [4k, 4k+4)
"""

import os
import sys

import numpy as np


def _ensure_paths():
    for p in ("/opt/trn_rl_repo", "/root/.axon_site/_ro/trn_rl_repo"):
        if os.path.isdir(p) and p not in sys.path:
            sys.path.append(p)
    import concourse.bass  # noqa: F401


_ensure_paths()

import concourse.bacc as bacc  # noqa: E402
import concourse.bass as bass  # noqa: E402
import concourse.tile as tile  # noqa: E402
from concourse import mybir  # noqa: E402
from concourse.bass_utils import run_bass_kernel_spmd  # noqa: E402
from concourse.masks import make_identity  # noqa: E402

S, B, D, H, R, V = 128, 32, 1024, 1024, 256, 32000
P = 128
NCORES = 8
VS = V // NCORES  # vocab shard per core (4000)
NVT = 8  # vocab N-tiles per core
VNT = VS // NVT  # 500 columns per vocab tile
F4R = 4 * R  # 1024
BF16 = mybir.dt.bfloat16
F32 = mybir.dt.float32
I32 = mybir.dt.int32

Sig = mybir.ActivationFunctionType.Sigmoid
Tanh = mybir.ActivationFunctionType.Tanh


def build_nc(nsteps=S):
    """Build the per-core Bass program (same program on all 8 cores)."""
    ntiles = nsteps // 4
    nc = bacc.Bacc("TRN2", target_bir_lowering=False, debug=False)

    # ---- DRAM I/O ----
    emb_d = nc.dram_tensor("emb", (V, D), BF16, kind="ExternalInput")
    idx_d = nc.dram_tensor("idx", (P, ntiles), I32, kind="ExternalInput")
    a_d = nc.dram_tensor("a_r", (8, P, F4R), BF16, kind="ExternalInput")
    b_d = nc.dram_tensor("b_r", (8, P, F4R), BF16, kind="ExternalInput")
    ct_d = nc.dram_tensor("ct_r", (8, P, H), BF16, kind="ExternalInput")
    w_d = nc.dram_tensor("wT_r", (8, P, VS), BF16, kind="ExternalInput")
    bias_d = nc.dram_tensor("bias_b", (P, VS), F32, kind="ExternalInput")
    h0f_d = nc.dram_tensor("h0f", (P, 256), F32, kind="ExternalInput")
    c0f_d = nc.dram_tensor("c0f", (P, 256), F32, kind="ExternalInput")
    h0e_d = nc.dram_tensor("h0tE", (P, P), BF16, kind="ExternalInput")
    h0o_d = nc.dram_tensor("h0tO", (P, P), BF16, kind="ExternalInput")

    out_d = nc.dram_tensor("out", (P * ntiles, VS), F32, kind="ExternalOutput")
    hT_d = nc.dram_tensor("hT_f", (P, 256), F32, kind="ExternalOutput")
    cT_d = nc.dram_tensor("cT_f", (P, 256), F32, kind="ExternalOutput")

    with tile.TileContext(nc) as tc:
        with (
            tc.tile_pool(name="const", bufs=1) as const,
            tc.tile_pool(name="state", bufs=1) as statep,
            tc.tile_pool(name="xpool", bufs=2) as xpool,
            tc.tile_pool(name="xbpool", bufs=2) as xbpool,
            tc.tile_pool(name="stepp", bufs=3) as stepp,
            tc.tile_pool(name="gtp", bufs=4) as gtp,
            tc.tile_pool(name="actp", bufs=2) as actp,
            tc.tile_pool(name="mrgp", bufs=4) as mrgp,
            tc.tile_pool(name="outp", bufs=3) as outp,
            tc.tile_pool(name="ps_g", bufs=1, space="PSUM") as ps_gp,
            tc.tile_pool(name="ps_gates", bufs=1, space="PSUM") as ps_gatesp,
            tc.tile_pool(name="ps_t", bufs=2, space="PSUM") as ps_tp,
            tc.tile_pool(name="ps_xb", bufs=1, space="PSUM") as ps_xbp,
            tc.tile_pool(name="ps_out", bufs=2, space="PSUM") as ps_outp,
        ):
            # ---- persistent SBUF loads ----
            a_sb = const.tile([P, 8 * F4R], BF16)
            b_sb = const.tile([P, 8 * F4R], BF16)
            ct_sb = const.tile([P, 8 * H], BF16)
            w_sb = const.tile([P, 8 * VS], BF16)
            bias_sb = const.tile([P, VS], F32)
            idx_sb = const.tile([P, ntiles], I32)
            ident = const.tile([P, P], BF16)
            for c in range(8):
                nc.sync.dma_start(out=a_sb[:, c * F4R:(c + 1) * F4R], in_=a_d[c])
                nc.sync.dma_start(out=b_sb[:, c * F4R:(c + 1) * F4R], in_=b_d[c])
                nc.sync.dma_start(out=ct_sb[:, c * H:(c + 1) * H], in_=ct_d[c])
                nc.sync.dma_start(out=w_sb[:, c * VS:(c + 1) * VS], in_=w_d[c])
            nc.sync.dma_start(out=bias_sb[:], in_=bias_d[:])
            nc.sync.dma_start(out=idx_sb[:], in_=idx_d[:])
            make_identity(nc, ident[:])

            c_st = statep.tile([P, 256], BF16)
            c0_sb = statep.tile([P, 256], F32)
            nc.sync.dma_start(out=c0_sb[:], in_=c0f_d[:])
            nc.vector.tensor_copy(c_st[:], c0_sb[:])
            h0_E = statep.tile([P, P], BF16)
            h0_O = statep.tile([P, P], BF16)
            nc.sync.dma_start(out=h0_E[:], in_=h0e_d[:])
            nc.sync.dma_start(out=h0_O[:], in_=h0o_d[:])

            xb_tiles = [None] * ntiles
            # per token-tile merged hidden-transpose buffers, written
            # strided by each step's eviction, consumed by MM1 + decode:
            # hTm[p, 128*jj + 32*t' + b] = hT_step(4k+t')[p, 32*jj + b]
            hTm_tiles = [None] * ntiles

            def lhsT_chunk(t, c):
                """hT chunk c of step t-1 (lhsT for MM1 at step t)."""
                jj = c // 2
                if t == 0:
                    return (h0_E if c % 2 == 0 else h0_O)[:, 32 * jj:32 * jj + 32]
                e, o = hTm_tiles[(t - 1) // 4]
                v = e if c % 2 == 0 else o
                return v[:, P * jj + 32 * ((t - 1) % 4):P * jj + 32 * ((t - 1) % 4) + 32]

            def phase_a(k, part):
                """Gather + transpose + x@b GEMM for token tile k.
                part 0: gather, transposes c=0..3, GEMM n2=0
                part 1: transposes c=4..7, GEMM n2=1."""
                if part == 0:
                    x_k = xpool.tile([P, D], BF16, tag="x")
                    xT_k = xpool.tile([P, D], BF16, tag="xT")
                    xb_k = xbpool.tile([P, F4R], BF16, tag="xb")
                    phase_a.cur = (x_k, xT_k, xb_k)
                    nc.gpsimd.indirect_dma_start(
                        out=x_k[:],
                        out_offset=None,
                        in_=emb_d[:],
                        in_offset=bass.IndirectOffsetOnAxis(
                            ap=idx_sb[:, k:k + 1], axis=0
                        ),
                    )
                x_k, xT_k, xb_k = phase_a.cur
                crange = range(0, 4) if part == 0 else range(4, 8)
                for c in crange:
                    pst = ps_tp.tile([P, P], BF16, tag="pst")
                    nc.tensor.transpose(
                        out=pst[:], in_=x_k[:, c * P:(c + 1) * P], identity=ident[:]
                    )
                    nc.vector.tensor_copy(xT_k[:, c * P:(c + 1) * P], pst[:])
                n2 = part
                ps_xb = ps_xbp.tile([P, 512], F32, tag="psxb")
                for c in range(8):
                    nc.tensor.matmul(
                        ps_xb[:],
                        lhsT=xT_k[:, c * P:(c + 1) * P],
                        rhs=b_sb[:, c * F4R + n2 * 512: c * F4R + (n2 + 1) * 512],
                        start=(c == 0),
                        stop=(c == 7),
                    )
                nc.vector.tensor_copy(xb_k[:, n2 * 512:(n2 + 1) * 512], ps_xb[:])
                if part == 1:
                    xb_tiles[k] = xb_k

            def scan_step(t):
                k, tl = t // 4, t % 4
                if tl == 0:
                    mE = mrgp.tile([P, 512], BF16, tag="hTmE")
                    mO = mrgp.tile([P, 512], BF16, tag="hTmO")
                    hTm_tiles[k] = (mE, mO)
                xb_k = xb_tiles[k]
                # fold xb for this step: (32,1024) -> B-fold (128,256)
                xbf = stepp.tile([P, 256], BF16, tag="xbf")
                for j in range(4):
                    nc.sync.dma_start(
                        out=xbf[32 * j:32 * (j + 1), :],
                        in_=xb_k[32 * tl:32 * (tl + 1), 256 * j:256 * (j + 1)],
                    )
                # MM1: ha, B-folded over 4R (gate quarters)
                ps_g = ps_gp.tile([P, 256], F32, tag="psg")
                for c in range(8):
                    lhsT = lhsT_chunk(t, c)
                    for j in range(4):
                        nc.tensor.matmul(
                            ps_g[32 * j:32 * (j + 1), :],
                            lhsT=lhsT,
                            rhs=a_sb[:, c * F4R + 256 * j: c * F4R + 256 * (j + 1)],
                            start=(c == 0),
                            stop=(c == 7),
                            tile_position=(0, 32 * j),
                            skip_group_check=True,
                        )
                # g = ha * xb (B-fold, bf16), halves pipelined into transposes
                g_sb = stepp.tile([P, 256], BF16, tag="gsb")
                gT_A = gtp.tile([P, P], BF16, tag="gTA")
                gT_B = gtp.tile([P, P], BF16, tag="gTB")
                for half, gt in ((0, gT_A), (1, gT_B)):
                    sl = slice(half * P, (half + 1) * P)
                    nc.vector.tensor_mul(g_sb[:, sl], ps_g[:, sl], xbf[:, sl])
                    pst = ps_tp.tile([P, P], BF16, tag="pst")
                    nc.tensor.transpose(
                        out=pst[:], in_=g_sb[:, sl], identity=ident[:]
                    )
                    nc.vector.tensor_copy(gt[:], pst[:])
                # MM2: gates, B-folded over H.  one psum tile [f|i|o|g]
                ps_gate = ps_gatesp.tile([P, 1024], F32, tag="psgate")
                gate_off = {0: 0, 1: 256, 3: 512, 2: 768}  # f,i,o,g
                for rc, gt in ((0, gT_A), (1, gT_B)):
                    for gk in range(4):
                        off = gate_off[gk]
                        lhsT = gt[:, 32 * gk:32 * (gk + 1)]
                        m = 2 * gk + rc
                        for j in range(4):
                            nc.tensor.matmul(
                                ps_gate[32 * j:32 * (j + 1), off:off + 256],
                                lhsT=lhsT,
                                rhs=ct_sb[:, m * H + 256 * j: m * H + 256 * (j + 1)],
                                start=(rc == 0),
                                stop=(rc == 1),
                                tile_position=(0, 32 * j),
                                skip_group_check=True,
                            )
                # activations: sigmoid(f,i) gates the c-chain; tanh(g) next;
                # sigmoid(o) runs under the DVE chain.
                sfi = actp.tile([P, 512], BF16, tag="sfi")
                tg = actp.tile([P, 256], BF16, tag="tg")
                so = actp.tile([P, 256], BF16, tag="so")
                nc.scalar.activation(sfi[:], ps_gate[:, 0:512], Sig)
                nc.scalar.activation(tg[:], ps_gate[:, 768:1024], Tanh)
                nc.scalar.activation(so[:], ps_gate[:, 512:768], Sig)
                # c' = f*c + i*tg ; h = o * tanh(c')   (bf16 state)
                t_ig = stepp.tile([P, 256], BF16, tag="tig")
                t_cf = stepp.tile([P, 256], BF16, tag="tcf")
                nc.vector.tensor_mul(t_cf[:], sfi[:, 0:256], c_st[:])
                nc.vector.tensor_mul(t_ig[:], sfi[:, 256:512], tg[:])
                nc.vector.tensor_add(c_st[:], t_ig[:], t_cf[:])
                tc_ = actp.tile([P, 256], BF16, tag="tc")
                nc.scalar.activation(tc_[:], c_st[:], Tanh)
                h_b = stepp.tile([P, 256], BF16, tag="hb")
                nc.vector.tensor_mul(h_b[:], so[:], tc_[:])
                # transpose h -> strided slots of this tile's merged buffers
                mE, mO = hTm_tiles[k]
                for half, dst in ((0, mE), (1, mO)):
                    pst = ps_tp.tile([P, P], BF16, tag="pst")
                    nc.tensor.transpose(
                        out=pst[:], in_=h_b[:, half * P:(half + 1) * P],
                        identity=ident[:],
                    )
                    dv = dst[:].rearrange("p (j t b) -> p j t b", j=4, t=4)
                    nc.vector.tensor_copy(
                        dv[:, :, tl, :], pst[:].rearrange("p (j b) -> p j b", j=4)
                    )
                if t == nsteps - 1:
                    h_f = stepp.tile([P, 256], F32, tag="hf")
                    nc.vector.tensor_mul(h_f[:], so[:], tc_[:])
                    nc.sync.dma_start(out=hT_d[:], in_=h_f[:])
                    c_f = stepp.tile([P, 256], F32, tag="cf")
                    nc.vector.tensor_copy(c_f[:], c_st[:])
                    nc.sync.dma_start(out=cT_d[:], in_=c_f[:])

            def decode_ntile(k, n):
                """Decoder GEMM for token M-tile k, one vocab N-tile n."""
                hTmE, hTmO = hTm_tiles[k]
                ps_o = ps_outp.tile([P, VNT], F32, tag="pso")
                for c in range(8):
                    v = hTmE if c % 2 == 0 else hTmO
                    lhsT = v[:, P * (c // 2):P * (c // 2) + P]
                    nc.tensor.matmul(
                        ps_o[:],
                        lhsT=lhsT,
                        rhs=w_sb[:, c * VS + VNT * n: c * VS + VNT * (n + 1)],
                        start=(c == 0),
                        stop=(c == 7),
                    )
                o_sb = outp.tile([P, VNT], F32, tag="osb")
                nc.vector.tensor_add(
                    o_sb[:], ps_o[:], bias_sb[:, VNT * n:VNT * (n + 1)]
                )
                nc.sync.dma_start(
                    out=out_d[P * k:P * (k + 1), VNT * n:VNT * (n + 1)],
                    in_=o_sb[:],
                )

            # ---- emission: interleave gather/xb, scan, decode ----
            phase_a(0, 0)
            phase_a(0, 1)
            for k in range(ntiles):
                for tl in range(4):
                    scan_step(4 * k + tl)
                    # filler work between steps to keep the PE busy
                    if tl == 0 and k + 1 < ntiles:
                        phase_a(k + 1, 0)
                    elif tl == 1 and k + 1 < ntiles:
                        phase_a(k + 1, 1)
                    if k >= 1:
                        for n in (2 * tl, 2 * tl + 1):
                            decode_ntile(k - 1, n)
            for n in range(NVT):
                decode_ntile(ntiles - 1, n)

    nc.compile()
    return nc


# ---------------- host side ----------------

def _bf16(x):
    import ml_dtypes
    return np.asarray(x, np.float32).astype(ml_dtypes.bfloat16)


def _prep_shared(inp, emb, a, b, ct, nsteps):
    ntiles = nsteps // 4
    idx = np.asarray(inp).astype(np.int32).reshape(S * B)[:P * ntiles]
    idx_h = np.ascontiguousarray(idx.reshape(ntiles, P).T)  # (128, ntiles)
    emb_h = np.ascontiguousarray(_bf16(emb))
    a_h = np.ascontiguousarray(_bf16(np.asarray(a)).reshape(8, P, F4R))
    b_h = np.ascontiguousarray(_bf16(np.asarray(b)).reshape(8, P, F4R))
    ct_h = np.ascontiguousarray(_bf16(np.asarray(ct)).reshape(8, P, H))
    return idx_h, emb_h, a_h, b_h, ct_h


def _fold(x):  # (32, 1024) -> (128, 256) B-fold
    return np.ascontiguousarray(
        x.reshape(B, 4, 256).transpose(1, 0, 2).reshape(P, 256)
    )


def _unfold(xf):  # inverse of _fold
    return np.ascontiguousarray(
        xf.reshape(4, B, 256).transpose(1, 0, 2).reshape(B, 4 * 256)
    )


def make_in_maps(inputs, nsteps=S, h0=None, c0=None):
    inp, emb = inputs["inp"], inputs["emb"]
    a, b, ct = inputs["a"], inputs["b"], inputs["ct"]
    dec_w, dec_b = np.asarray(inputs["dec_w"]), np.asarray(inputs["dec_b"])
    idx_h, emb_h, a_h, b_h, ct_h = _prep_shared(inp, emb, a, b, ct, nsteps)
    h0 = np.zeros((B, H), np.float32) if h0 is None else np.asarray(h0, np.float32)
    c0 = np.zeros((B, H), np.float32) if c0 is None else np.asarray(c0, np.float32)
    h0f, c0f = _fold(h0), _fold(c0)
    h0tE = np.ascontiguousarray(_bf16(h0f[:, :P].T))
    h0tO = np.ascontiguousarray(_bf16(h0f[:, P:].T))
    in_maps = []
    for i in range(NCORES):
        wsh = dec_w[i * VS:(i + 1) * VS]  # (4000, 1024)
        wT = np.ascontiguousarray(_bf16(wsh.T).reshape(8, P, VS))
        bias = np.ascontiguousarray(
            np.broadcast_to(dec_b[i * VS:(i + 1) * VS].astype(np.float32), (P, VS))
        )
        in_maps.append({
            "emb": emb_h, "idx": idx_h, "a_r": a_h, "b_r": b_h, "ct_r": ct_h,
            "wT_r": wT, "bias_b": bias, "h0f": h0f, "c0f": c0f,
            "h0tE": h0tE, "h0tO": h0tO,
        })
    return in_maps


_NC_CACHE = {}


def run_device(inputs, nsteps=S, h0=None, c0=None, trace=False):
    if nsteps not in _NC_CACHE:
        _NC_CACHE[nsteps] = build_nc(nsteps)
    nc = _NC_CACHE[nsteps]
    in_maps = make_in_maps(inputs, nsteps, h0, c0)
    run_bass_kernel_spmd(nc, in_maps, core_ids=list(range(NCORES)))  # warmup
    res = run_bass_kernel_spmd(
        nc, in_maps, core_ids=list(range(NCORES)), trace=trace
    )
    out = np.concatenate(
        [res.results[i]["out"] for i in range(NCORES)], axis=1
    ).reshape(nsteps, B, V)
    h_T = _unfold(res.results[0]["hT_f"])
    c_T = _unfold(res.results[0]["cT_f"])
    return out, (c_T, h_T), res


def kernel(**inputs):
    out, (c_T, h_T), _ = run_device(inputs, nsteps=S)
    return out.astype(np.float32), (c_T.astype(np.float32), h_T.astype(np.float32))
